# revision 1
# baseline (speedup 1.0000x reference)
"""Tensor-parallel multi-head attention for Trainium2 (8 NeuronCores).

Problem: B=2, T=2048, E=1024, H=16 heads of dim 64.
  q/k/v = einsum('hei,bte->hbti'); s = q@k^T/sqrt(T); p = softmax(s)
  att = p@v; out = concat_heads(att) @ Wo^T

Sharding: tensor-parallel over heads - 2 heads per core. Each core computes
its heads' attention plus its slice of the output projection (Wo sharded
along its input axis); partial outputs are summed across cores.

Numerics: all-fp16 storage/matmuls with f32 PSUM accumulation measures
7.5e-3 rel err in a bit-exact numpy model (gate is 2e-2). Softmax logits
reach +-1504 after the 1/sqrt(T) scale (the reference scales by sqrt(T),
not sqrt(head_dim)), so a per-row max subtraction is mandatory; exp outputs
(unnormalized, in (0,1]) are stored fp16 and the 1/den normalization is
applied to OT afterwards via a PE broadcast of per-token reciprocals.

Scheduling: a single 8-slot one-bank PSUM ring ([128,512] f32 tiles) keeps
~2 query blocks of softmax in flight. Per query block (128 tokens): one
Ldweights + 4 QK matmuls; 4 DVE max-reduces; 4 ACT exps (bias=-max); one
DMA transpose of P into s-major layout. PV of the previous head runs as two
2-slot bursts (at block 3 and 9 of the next head) so its accumulators pin
ring slots only briefly. Default build is the "vaug" variant: V carries a
ones column per head, so the PV matmul's 65th output row produces den for
free (no ACT accumulator reads); den hops PSUM->SBUF same-partition, then
an SBUF->SBUF DMA lands it on partition 0 for the reciprocal + broadcast.
"""

import sys

sys.path.insert(0, "/opt/trn_rl_repo")

import numpy as np

import concourse.bass as bass
import concourse.mybir as mybir
import concourse.tile as tile
from concourse import bacc

NF16 = np.float16

B, T, E = 2, 2048, 1024
H, I = 16, 64
NCORES = 8
HPC = H // NCORES            # heads per core = 2
BT = B * T                   # 4096
HI = HPC * I                 # 128 = per-core slice of the h*i axis
EC = E // 128                # 8 e-chunks
NTB = T // 128               # 16 query blocks per batch
SCALE = 1.0 / float(np.sqrt(np.float32(T)))

F32 = mybir.dt.float32
FP16 = mybir.dt.float16
AX = mybir.AxisListType.X
MUL = mybir.AluOpType.mult
ADD = mybir.AluOpType.add
EXP = mybir.ActivationFunctionType.Exp


ABLATE_DEFAULT = "vaug"   # den via ones-column in V (see emit_pv_burst)


def build_program(repeat: int = 1, ablate: str | None = None) -> bass.Bass:
    if ablate is None:
        ablate = ABLATE_DEFAULT
    nc = bacc.Bacc("TRN2", target_bir_lowering=False, debug=False,
                   num_devices=NCORES)

    xt_d = nc.dram_tensor("xt", [E, BT], FP16, kind="ExternalInput")
    wq_d = nc.dram_tensor("wq", [128, EC, HI], FP16, kind="ExternalInput")
    wk_d = nc.dram_tensor("wk", [128, EC, HI], FP16, kind="ExternalInput")
    wv_d = nc.dram_tensor("wv", [128, EC, HI], FP16, kind="ExternalInput")
    wo_d = nc.dram_tensor("wo_t", [HI, E], FP16, kind="ExternalInput")
    id_d = nc.dram_tensor("ident", [128, 128], FP16, kind="ExternalInput")
    on_d = nc.dram_tensor("ones64", [1, 64], FP16, kind="ExternalInput")
    out_d = nc.dram_tensor("out", [BT, E], FP16, kind="ExternalOutput")
    if "dump" in ablate:
        vd_d = nc.dram_tensor("vdump", [128, BT // 128, 130], FP16,
                              kind="ExternalOutput")
        od_d = nc.dram_tensor("otdump", [128, BT], FP16,
                              kind="ExternalOutput")
        rd_d = nc.dram_tensor("rowdump", [4, T], FP16, kind="ExternalOutput")

    with tile.TileContext(nc) as tc:
        with (
            tc.tile_pool(name="psp", bufs=8, space="PSUM") as psp,
            tc.tile_pool(name="wp", bufs=1) as wp,
            tc.tile_pool(name="xp", bufs=3) as xp,
            tc.tile_pool(name="pk", bufs=1) as pk,
            tc.tile_pool(name="big", bufs=2) as bigp,
            tc.tile_pool(name="ptp",
                         bufs=4 if "ptp4" in ablate else 3) as ptp,
            tc.tile_pool(name="stp", bufs=2) as stp,
        ):
            wq = wp.tile([128, EC, HI], FP16, tag="wq")
            wk = wp.tile([128, EC, HI], FP16, tag="wk")
            wv = wp.tile([128, EC, HI], FP16, tag="wv")
            wo = wp.tile([128, E], FP16, tag="wo")
            ident = wp.tile([128, 128], FP16, tag="ident")
            ones64 = wp.tile([1, 64], FP16, tag="ones64")
            nc.gpsimd.dma_start(wq[:], wq_d[:])
            nc.gpsimd.dma_start(wk[:], wk_d[:])
            nc.gpsimd.dma_start(wv[:], wv_d[:])
            nc.gpsimd.dma_start(wo[:], wo_d[:])
            nc.gpsimd.dma_start(ident[:], id_d[:])
            nc.gpsimd.dma_start(ones64[:], on_d[:])

            vaug = "vaug" in ablate
            for _rep in range(repeat):
                QT = pk.tile([128, BT], FP16, tag="QT")
                KT = pk.tile([128, BT], FP16, tag="KT")
                # vaug: V gets a ones column per head (col 64 / 129) so the PV
                # matmul's 65th output row accumulates den = sum_s P[t,s].
                V = pk.tile([128, BT // 128, 130 if vaug else HI], FP16,
                            tag="V")
                OT = pk.tile([128, BT], FP16, tag="OT")
                if vaug:
                    nc.vector.memset(V[:, :, 64:65], 1.0)
                    nc.vector.memset(V[:, :, 129:130], 1.0)

                # ---------- Phase 1: QKV projections (1024-token pairs) -------
                def emit_phase1(bp):
                    cols = slice(bp * 1024, (bp + 1) * 1024)
                    xcs = []
                    for g in range(4):
                        xg = xp.tile([128, 2, 1024], FP16, tag="x",
                                     name=f"x{bp}_{g}")
                        nc.gpsimd.dma_start(
                            xg[:], xt_d[g * 256:(g + 1) * 256, cols].rearrange(
                                "(o p) t -> p o t", p=128))
                        xcs.append(xg)
                    ps6 = {}
                    for nm in ("q0", "q1", "k0", "k1", "v0", "v1"):
                        ps6[nm] = psp.tile([128, 512], F32, tag="ps",
                                           name=f"{nm}_{bp}")
                    for ec in range(EC):
                        xc = xcs[ec // 2][:, ec % 2, :]
                        st = (ec == 0)
                        sp = (ec == EC - 1)
                        for w_, nm in ((wq, "q"), (wk, "k"), (wv, "v")):
                            for half in range(2):
                                nc.tensor.matmul(
                                    ps6[f"{nm}{half}"][:], w_[:, ec, :],
                                    xc[:, half * 512:(half + 1) * 512],
                                    start=st, stop=sp)
                    # evac: Q scaled by 1/sqrt(T) on ACT; K on DVE; V on ACT
                    for half in range(2):
                        hcols = slice(bp * 1024 + half * 512,
                                      bp * 1024 + (half + 1) * 512)
                        nc.scalar.mul(QT[:, hcols], ps6[f"q{half}"][:], SCALE)
                        nc.vector.tensor_copy(KT[:, hcols], ps6[f"k{half}"][:])
                    vsb = ptp.tile([128, 1024], FP16, tag="Pt", name=f"vsb{bp}")
                    nc.scalar.copy(vsb[:, 0:512], ps6["v0"][:])
                    nc.scalar.copy(vsb[:, 512:1024], ps6["v1"][:])
                    if vaug:
                        # xbar transpose needs a contiguous dst; bounce via a
                        # staging tile, then engine-copy into the strided V
                        # layout (64 data cols + ones col per head)
                        stag = ptp.tile([128, 8, 128], FP16, tag="Pt",
                                        name=f"stag{bp}")
                        nc.sync.dma_start_transpose(stag[:], vsb[:])
                        nc.vector.tensor_copy(
                            V[:, bp * 8:(bp + 1) * 8, 0:64],
                            stag[:, :, 0:64])
                        nc.vector.tensor_copy(
                            V[:, bp * 8:(bp + 1) * 8, 65:129],
                            stag[:, :, 64:128])
                    else:
                        nc.sync.dma_start_transpose(
                            V[:, bp * 8:(bp + 1) * 8, :], vsb[:])

                for bp in range(4):
                    emit_phase1(bp)

                # ---------- Phase 2 + PV bursts / normalize / output ----------
                def emit_pv_burst(prev, half):
                    # PV of prev head for t-cols half*1024:(half+1)*1024:
                    # 2 ring slots, 16 Ldweights + 32 matmuls, then ACT evac
                    # of the unnormalized OT columns. With vaug the lhsT is 65
                    # wide (ones col) and PSUM row 64 is den for these tokens.
                    pb, ph = prev["b"], prev["h"]
                    hr = slice(ph * 64, (ph + 1) * 64)
                    vw = 65 if vaug else 64
                    vcols = slice(ph * vw, (ph + 1) * vw)
                    o0 = psp.tile([128, 512], F32, tag="ps",
                                  name=f"o0_{pb}{ph}{half}")
                    o1 = psp.tile([128, 512], F32, tag="ps",
                                  name=f"o1_{pb}{ph}{half}")
                    for sc in range(NTB):
                        lhs = V[:, pb * NTB + sc, vcols]
                        st = (sc == 0)
                        sp = (sc == NTB - 1)
                        nc.tensor.matmul(
                            o0[0:vw, :], lhs,
                            prev["PT"][:, sc, half * 1024:half * 1024 + 512],
                            start=st, stop=sp)
                        nc.tensor.matmul(
                            o1[0:vw, :], lhs,
                            prev["PT"][:, sc, half * 1024 + 512:(half + 1) * 1024],
                            start=st, stop=sp)
                    c0 = slice(pb * T + half * 1024, pb * T + half * 1024 + 512)
                    c1 = slice(pb * T + half * 1024 + 512,
                               pb * T + (half + 1) * 1024)
                    nc.scalar.copy(OT[hr, c0], o0[0:64, :])
                    nc.scalar.copy(OT[hr, c1], o1[0:64, :])
                    if vaug:
                        if half == 0:
                            prev["vrow"] = stp.tile([1, T], FP16, tag="vrow",
                                                    bufs=3,
                                                    name=f"vrow{pb}{ph}")
                        vrow = prev["vrow"]
                        # engines can't cross partitions: copy den (PSUM row
                        # 64) to SBUF on the same partition, then DMA it to
                        # vrow's partition 0
                        dst = stp.tile([128, 1024], FP16, tag="dstage",
                                       bufs=1, name=f"ds{pb}{ph}{half}")
                        nc.scalar.copy(dst[64:65, 0:512], o0[64:65, :])
                        nc.scalar.copy(dst[64:65, 512:1024], o1[64:65, :])
                        nc.gpsimd.dma_start(
                            vrow[0:1, half * 1024:(half + 1) * 1024],
                            dst[64:65, :])
                        if half == 1:
                            with nc.allow_low_precision(
                                    reason="fp16 reciprocal of den >= 1"):
                                nc.vector.reciprocal(vrow[0:1, :], vrow[0:1, :])
                            rows[(pb, ph)] = vrow

                def emit_normalize_b(pb, rows):
                    # OT[:, pb] *= bcast(rcp): both heads at once per
                    # 512-column chunk; bc rows 0:64 <- rcp_h0, 64:128 <- rcp_h1
                    for ck in range(4):
                        cs = slice(ck * 512, (ck + 1) * 512)
                        ocols = slice(pb * T + ck * 512, pb * T + (ck + 1) * 512)
                        bc = psp.tile([128, 512], F32, tag="ps",
                                      name=f"bc{pb}{ck}")
                        nc.tensor.matmul(bc[0:64, :], ones64[0:1, :],
                                         rows[0][0:1, cs], start=True,
                                         stop=True)
                        nc.tensor.matmul(bc[64:128, :], ones64[0:1, :],
                                         rows[1][0:1, cs], start=True,
                                         stop=True)
                        nc.vector.tensor_tensor(OT[:, ocols], bc[:],
                                                OT[:, ocols], MUL)

                def emit_phase3_block(b, ob):
                    trows = slice(b * T + ob * 128, b * T + (ob + 1) * 128)
                    w0 = psp.tile([128, 512], F32, tag="ps", name=f"w0_{b}{ob}")
                    w1 = psp.tile([128, 512], F32, tag="ps", name=f"w1_{b}{ob}")
                    nc.tensor.matmul(w0[:], OT[:, trows], wo[:, 0:512],
                                     start=True, stop=True)
                    nc.tensor.matmul(w1[:], OT[:, trows], wo[:, 512:1024],
                                     start=True, stop=True)
                    osb = ptp.tile([128, 1024], FP16, tag="Pt",
                                   name=f"osb{b}_{ob}")
                    if ob % 2:
                        nc.scalar.copy(osb[:, 0:512], w0[:])
                        nc.vector.tensor_copy(osb[:, 512:1024], w1[:])
                    else:
                        nc.vector.tensor_copy(osb[:, 0:512], w0[:])
                        nc.scalar.copy(osb[:, 512:1024], w1[:])
                    q_ = nc.sync if ob % 2 else nc.scalar
                    q_.dma_start(out_d[trows, :], osb[:])

                prev = None          # head whose PV bursts run during cur QK
                rows = {}            # (b, h) -> rcp row tile
                phase3_todo = []
                for b in range(B):
                    for h in range(HPC):
                        hr = slice(h * 64, (h + 1) * 64)
                        PT = bigp.tile([128, NTB, T], FP16, tag="PT",
                                       name=f"PT{b}{h}")
                        Den = None
                        if not vaug:
                            Den = stp.tile([128, 4 * NTB], F32, tag="Den",
                                           name=f"Den{b}{h}")
                            if "noden" in ablate:
                                nc.vector.memset(Den[:], 1.0)
                        for tb in range(NTB):
                            bt0 = 2 if "b28" in ablate else 3
                            if prev is not None and tb == bt0:
                                emit_pv_burst(prev, 0)
                            if prev is not None and tb == bt0 + 6:
                                emit_pv_burst(prev, 1)
                                if prev["h"] == HPC - 1:
                                    emit_normalize_b(
                                        prev["b"],
                                        (rows[(prev["b"], 0)],
                                         rows[(prev["b"], 1)]))
                                    phase3_todo.extend(
                                        (prev["b"], ob) for ob in range(NTB))
                            tcols = slice(b * T + tb * 128,
                                          b * T + (tb + 1) * 128)
                            Ss = []
                            for j in range(4):
                                S = psp.tile([128, 512], F32, tag="ps",
                                             name=f"S{b}{h}{tb}{j}")
                                scols = slice(b * T + j * 512,
                                              b * T + (j + 1) * 512)
                                nc.tensor.matmul(S[:], QT[hr, tcols],
                                                 KT[hr, scols], start=True,
                                                 stop=True)
                                Ss.append(S)
                            # per-row max over the 4 S tiles
                            mp = stp.tile([128, 4], F32, tag="mp", bufs=4,
                                          name=f"mp{b}{h}{tb}")
                            for j in range(4):
                                nc.vector.reduce_max(mp[:, j:j + 1], Ss[j][:],
                                                     axis=AX)
                            negb = stp.tile([128, 1], F32, tag="negb", bufs=4,
                                            name=f"negb{b}{h}{tb}")
                            nc.vector.reduce_max(negb[:], mp[:], axis=AX,
                                                 negate=True)
                            # exp (unnormalized) + den accumulation
                            Pt = ptp.tile([128, 2048], FP16, tag="Pt",
                                          name=f"Pt{b}{h}{tb}")
                            for j in range(4):
                                nc.scalar.activation(
                                    Pt[:, j * 512:(j + 1) * 512], Ss[j][:],
                                    EXP, bias=negb[:], scale=1.0,
                                    accum_out=(
                                        None
                                        if (vaug or "noden" in ablate) else
                                        Den[:, 4 * tb + j:4 * tb + j + 1]))
                            tq = (nc.scalar if ("tsplit" in ablate and tb % 2)
                                  else nc.sync)
                            tq.dma_start_transpose(
                                PT[:, :, tb * 128:(tb + 1) * 128], Pt[:])
                            if phase3_todo:
                                emit_phase3_block(*phase3_todo.pop(0))
                        # ---- den -> rcp row for this head ----
                        if vaug:
                            prev = {"b": b, "h": h, "PT": PT}
                            continue
                        d2a = stp.tile([128, NTB], F32, tag="d2a",
                                       name=f"d2a{b}{h}")
                        d2b = stp.tile([128, NTB], F32, tag="d2b",
                                       name=f"d2b{b}{h}")
                        nc.vector.tensor_tensor(d2a[:], Den[:, 0::4],
                                                Den[:, 1::4], ADD)
                        nc.vector.tensor_tensor(d2b[:], Den[:, 2::4],
                                                Den[:, 3::4], ADD)
                        den16 = stp.tile([128, NTB], F32, tag="den16",
                                         name=f"d16_{b}{h}")
                        nc.vector.tensor_tensor(den16[:], d2a[:], d2b[:], ADD)
                        rcp32 = stp.tile([128, NTB], F32, tag="rcp32",
                                         name=f"r32_{b}{h}")
                        nc.vector.reciprocal(rcp32[:], den16[:])
                        rcp16 = stp.tile([128, NTB], FP16, tag="rcp16",
                                         name=f"r16_{b}{h}")
                        nc.vector.tensor_copy(rcp16[:], rcp32[:])
                        rcpT = psp.tile([16, 128], FP16, tag="ps",
                                        name=f"rT{b}{h}")
                        nc.tensor.transpose(rcpT[:], rcp16[:], ident[:])
                        rcpT_sb = stp.tile([16, 128], FP16, tag="rcpTsb",
                                           name=f"rTs{b}{h}")
                        nc.vector.tensor_copy(rcpT_sb[:], rcpT[:])
                        row = stp.tile([1, T], FP16, tag="row",
                                       name=f"row{b}{h}")
                        nc.gpsimd.dma_start(
                            row[0:1, :].rearrange("o (a c) -> o a c", a=16),
                            rcpT_sb[:])
                        rows[(b, h)] = row
                        prev = {"b": b, "h": h, "PT": PT}

                # flush: PV + normalize of the last head, then remaining out
                emit_pv_burst(prev, 0)
                for _ in range(3):
                    if phase3_todo:
                        emit_phase3_block(*phase3_todo.pop(0))
                emit_pv_burst(prev, 1)
                emit_normalize_b(prev["b"],
                                 (rows[(prev["b"], 0)], rows[(prev["b"], 1)]))
                phase3_todo.extend((prev["b"], ob) for ob in range(NTB))
                for blk in phase3_todo:
                    emit_phase3_block(*blk)
                phase3_todo = []
                if "dump" in ablate:
                    nc.gpsimd.dma_start(vd_d[:], V[:])
                    nc.gpsimd.dma_start(od_d[:], OT[:])
                    for i, key in enumerate(((0, 0), (0, 1), (1, 0), (1, 1))):
                        nc.gpsimd.dma_start(rd_d[i:i + 1, :],
                                            rows[key][0:1, :])

    nc.compile()
    return nc


def make_in_maps(x, Wq, Wk, Wv, Wo):
    """Build the 8 per-core input maps from the full inputs."""
    x = np.asarray(x, np.float32)
    Wq = np.asarray(Wq, np.float32)
    Wk = np.asarray(Wk, np.float32)
    Wv = np.asarray(Wv, np.float32)
    Wo = np.asarray(Wo, np.float32)

    xt = np.ascontiguousarray(x.reshape(BT, E).T).astype(NF16)   # [E, BT]
    ident = np.eye(128, dtype=NF16)
    ones64 = np.ones((1, 64), dtype=NF16)
    in_maps = []
    for c in range(NCORES):
        hsl = slice(c * HPC, (c + 1) * HPC)

        def _pmaj(w):  # [E, HI] -> [128, EC, HI] (partition-major)
            return np.ascontiguousarray(
                w.reshape(EC, 128, HI).transpose(1, 0, 2)).astype(NF16)

        m = {
            "xt": xt,
            "wq": _pmaj(np.concatenate(list(Wq[hsl]), axis=1)),
            "wk": _pmaj(np.concatenate(list(Wk[hsl]), axis=1)),
            "wv": _pmaj(np.concatenate(list(Wv[hsl]), axis=1)),
            "wo_t": np.ascontiguousarray(
                Wo[:, c * HI:(c + 1) * HI].T).astype(NF16),
            "ident": ident,
            "ones64": ones64,
        }
        in_maps.append(m)
    return in_maps


_CACHED = {}


def _get_program() -> bass.Bass:
    if "p" not in _CACHED:
        _CACHED["p"] = build_program()
    return _CACHED["p"]


def kernel(**inputs) -> np.ndarray:
    from concourse.bass_utils import run_bass_kernel_spmd

    nc = _get_program()
    in_maps = make_in_maps(inputs["x"], inputs["Wq"], inputs["Wk"],
                           inputs["Wv"], inputs["Wo"])
    res = run_bass_kernel_spmd(nc, in_maps, core_ids=list(range(NCORES)))
    out = np.zeros((BT, E), np.float32)
    for c in range(NCORES):
        out += np.asarray(res.results[c]["out"], np.float32)
    return out.reshape(B, T, E)



# revision 35
# speedup vs baseline: 1.6446x; 1.6446x over previous
"""Tensor-parallel multi-head attention for Trainium2 (8 NeuronCores).

Problem: B=2, T=2048, E=1024, H=16 heads of dim 64.
  q/k/v = einsum('hei,bte->hbti'); s = q@k^T/sqrt(T); p = softmax(s)
  att = p@v; out = concat_heads(att) @ Wo^T

Sharding: tensor-parallel over heads - 2 heads per core. Each core computes
its heads' attention plus its slice of the output projection (Wo sharded
along its input axis); partial outputs are summed across cores.

v2 layout (vs the v1 8x[128,512] ring design):
  - ONE unified PSUM ring: 4 slots x [128,1024] (2 banks each) shared by
    the S tiles, the PV burst accumulators, and whole phase-3 blocks, so
    consecutive query blocks pipeline without a dedicated-bank squeeze.
  - Per query block: 4 QK matmuls into two S halves; row max = two
    negated DVE reduces (a DVE op may read only ONE PSUM operand) + a
    tiny min-combine; exp = 2x[128,1024] ACT instructions, each followed
    by its own [128,1024] DMA transpose into PT (s-major).
  - PV is reoriented: out[t, i] accumulates over the 16 s-chunks with
    lhsT = PT tiles (stationary) and rhs = V [s,65] (64 data cols + a
    ones column -> den lands per-partition-t at col 64 of the same PSUM
    region; 4 query blocks' accumulators pack into one bank, riding a
    single PSUM zero-region group: only the first matmul starts it).
  - Normalization on the evac: ACT Copy with per-partition scale=1/den
    into A2 [t, (h i)] tiles; one [128,128] DMA transpose per query
    block assembles OT [hi, t] for the out projection.
  - Phase 3 = whole [128,1024] blocks in one ring slot (2 matmuls,
    1 evac, 1 out DMA); the last head's PV bursts run inside its own
    softmax loop so the tail flush is just one burst + 4 blocks.
  - All DMAs issue from the SP HWDGE queue. Pool/GPSIMD is avoided:
    it cannot touch PSUM, and strided SBUF copies/memsets on it
    miscompute on real HW (sim-only success).

Engine budget per core-iteration (cost model): DVE 185 (row maxes +
QT/KT evacs), DMA 175 (64 P transposes = 115), ACT 170 (128 exps + A2
evacs), PE 139 (QKV 41 + QK 55 + PV 28 + out-proj 14).
"""

import sys

sys.path.insert(0, "/opt/trn_rl_repo")

import numpy as np

import concourse.bass as bass
import concourse.mybir as mybir
import concourse.tile as tile
from concourse import bacc

NF16 = np.float16

B, T, E = 2, 2048, 1024
H, I = 16, 64
NCORES = 8
HPC = H // NCORES            # heads per core = 2
BT = B * T                   # 4096
HI = HPC * I                 # 128 = per-core slice of the h*i axis
EC = E // 128                # 8 e-chunks
NTB = T // 128               # 16 query blocks per batch
SCALE = 1.0 / float(np.sqrt(np.float32(T)))

F32 = mybir.dt.float32
FP16 = mybir.dt.float16
AX = mybir.AxisListType.X
MUL = mybir.AluOpType.mult
ADD = mybir.AluOpType.add
MAX = mybir.AluOpType.max
EXP = mybir.ActivationFunctionType.Exp

ABLATE_DEFAULT = ""


def build_program(repeat: int = 1, ablate: str | None = None) -> bass.Bass:
    if ablate is None:
        ablate = ABLATE_DEFAULT
    nc = bacc.Bacc("TRN2", target_bir_lowering=False, debug=False,
                   num_devices=NCORES)

    xt_d = nc.dram_tensor("xt", [E, BT], FP16, kind="ExternalInput")
    wq_d = nc.dram_tensor("wq", [128, EC, HI], FP16, kind="ExternalInput")
    wk_d = nc.dram_tensor("wk", [128, EC, HI], FP16, kind="ExternalInput")
    wv_d = nc.dram_tensor("wv", [128, EC, HI], FP16, kind="ExternalInput")
    wo_d = nc.dram_tensor("wo_t", [HI, E], FP16, kind="ExternalInput")
    out_d = nc.dram_tensor("out", [BT, E], FP16, kind="ExternalOutput")

    with tile.TileContext(nc) as tc:
        with (
            tc.tile_pool(name="sp", bufs=4, space="PSUM") as spool,
            tc.tile_pool(name="wp", bufs=1) as wp,
            tc.tile_pool(name="xp", bufs=2) as xp,
            tc.tile_pool(name="pk", bufs=1) as pk,
            tc.tile_pool(name="big", bufs=2) as bigp,
            tc.tile_pool(name="ptp", bufs=3) as ptp,
            tc.tile_pool(name="a2p", bufs=2) as a2p,
            tc.tile_pool(name="stp", bufs=2) as stp,
        ):
            wq = wp.tile([128, EC, HI], FP16, tag="wq")
            wk = wp.tile([128, EC, HI], FP16, tag="wk")
            wv = wp.tile([128, EC, HI], FP16, tag="wv")
            wo = wp.tile([128, E], FP16, tag="wo")
            nc.sync.dma_start(wq[:], wq_d[:])
            nc.sync.dma_start(wk[:], wk_d[:])
            nc.sync.dma_start(wv[:], wv_d[:])
            nc.sync.dma_start(wo[:], wo_d[:])

            for _rep in range(repeat):
                QT = pk.tile([128, BT], FP16, tag="QT")
                KT = pk.tile([128, BT], FP16, tag="KT")
                # V: per s-chunk, 64 data cols + ones col per head (col 64 /
                # col 129): the PV matmul streams V as rhs and the ones col
                # accumulates den = sum_s P[t,s] into the same PSUM tile.
                V = pk.tile([128, BT // 128, 130], FP16, tag="V")
                OT = pk.tile([128, BT], FP16, tag="OT")
                nc.vector.memset(V[:, :, 64:65], 1.0)
                nc.vector.memset(V[:, :, 129:130], 1.0)

                # ---------- Phase 1: QKV projections (1024-token pairs) -----
                x_held = {}

                def load_x(bp):
                    cols = slice(bp * 1024, (bp + 1) * 1024)
                    xcs = []
                    for g in range(4):
                        xg = xp.tile([128, 2, 1024], FP16, tag="x",
                                     name=f"x{bp}_{g}")
                        nc.sync.dma_start(
                            xg[:], xt_d[g * 256:(g + 1) * 256, cols].rearrange(
                                "(o p) t -> p o t", p=128))
                        xcs.append(xg)
                    x_held[bp] = xcs
                    return xcs

                def emit_proj(bp, which):
                    xcs = x_held.get(bp) or load_x(bp)
                    pss = [(w_, spool.tile([128, 1024], F32, tag="S",
                                           name=f"p{nm}{bp}"))
                           for w_, nm in which]
                    for ec in range(EC):
                        xc = xcs[ec // 2][:, ec % 2, :]
                        st = (ec == 0)
                        sp = (ec == EC - 1)
                        for w_, ps in pss:
                            for half in range(2):
                                nc.tensor.matmul(
                                    ps[:, half * 512:(half + 1) * 512],
                                    w_[:, ec, :],
                                    xc[:, half * 512:(half + 1) * 512],
                                    start=st, stop=sp)
                    hcols = slice(bp * 1024, (bp + 1) * 1024)
                    for w_, ps in pss:
                        if w_ is wq:
                            nc.vector.tensor_scalar_mul(QT[:, hcols], ps[:],
                                                        SCALE)
                        elif w_ is wk:
                            nc.vector.tensor_copy(KT[:, hcols], ps[:])
                        else:
                            vsb = ptp.tile([128, 1024], FP16, tag="Pt",
                                           name=f"vsb{bp}")
                            nc.scalar.copy(vsb[:], ps[:])
                            stag = ptp.tile([128, 8, 128], FP16, tag="Pt",
                                            name=f"stag{bp}")
                            nc.sync.dma_start_transpose(stag[:], vsb[:])
                            nc.vector.tensor_copy(
                                V[:, bp * 8:(bp + 1) * 8, 0:64],
                                stag[:, :, 0:64])
                            nc.vector.tensor_copy(
                                V[:, bp * 8:(bp + 1) * 8, 65:129],
                                stag[:, :, 64:128])

                def emit_phase1(bp):
                    emit_proj(bp, ((wq, "q"), (wk, "k"), (wv, "v")))
                    x_held.pop(bp, None)

                # batch 0's tokens (bp 0,1) upfront; bp 2,3 interleave into
                # the early query blocks of (b=0, h=0)
                emit_phase1(0)
                emit_phase1(1)

                # ---------- Phase 2 + PV bursts / phase 3 -------------------
                def emit_pv_burst(prev, g):
                    # PV of prev head for query blocks 4g..4g+3, reoriented:
                    # out[t,i] accumulates over the 16 s-chunks; lhsT = PT
                    # tile [s,t-block], rhs = V [s, 65] (ones col -> den at
                    # col 64). One PSUM bank holds 4 blocks' accumulators.
                    pb, ph = prev["b"], prev["h"]
                    vw = 65
                    vcols = slice(ph * vw, (ph + 1) * vw)
                    ob = spool.tile([128, 4, vw], F32, tag="S",
                                    name=f"ob{pb}{ph}{g}")
                    # start=True zeroes the whole 2KB PSUM zero region (the
                    # bank), so only the first matmul starts the group and
                    # only the last one stops it; the 4 packed accumulator
                    # regions ride the same group.
                    for sc in range(NTB):
                        rhs = V[:, pb * NTB + sc, vcols]
                        for j in range(4):
                            tb = 4 * g + j
                            nc.tensor.matmul(
                                ob[:, j, :],
                                prev["PT"][:, sc, tb * 128:(tb + 1) * 128],
                                rhs, start=(sc == 0 and j == 0),
                                stop=(sc == NTB - 1 and j == 3))
                    # reciprocal of the 4 dens (strided col 64), then evac
                    # with per-partition normalize into A2[t, ph*64+i]
                    rcp = stp.tile([128, 4], F32, tag="rcp", bufs=3,
                                   name=f"rcp{pb}{ph}{g}")
                    nc.vector.reciprocal(rcp[:], ob[:, :, 64])
                    for j in range(4):
                        tb = 4 * g + j
                        A2 = prev["A2"][tb]
                        # normalize on the evac: ACT Copy with per-partition
                        # scale = 1/den
                        nc.scalar.activation(
                            A2[:, ph * 64:(ph + 1) * 64], ob[:, j, 0:64],
                            mybir.ActivationFunctionType.Copy,
                            scale=rcp[:, j:j + 1])
                        if ph == HPC - 1:
                            # both heads done for this query block: OT cols
                            nc.sync.dma_start_transpose(
                                OT[:, pb * T + tb * 128:pb * T + (tb + 1) * 128],
                                A2[:])

                def emit_phase3_block_spool(b, ob):
                    # whole [128,1024] block in one 2-bank ring slot
                    # (2 matmuls, 1 evac, 1 out DMA)
                    trows = slice(b * T + ob * 128, b * T + (ob + 1) * 128)
                    w = spool.tile([128, 1024], F32, tag="S",
                                   name=f"wf{b}_{ob}")
                    for half in range(2):
                        nc.tensor.matmul(w[:, half * 512:(half + 1) * 512],
                                         OT[:, trows],
                                         wo[:, half * 512:(half + 1) * 512],
                                         start=True, stop=True)
                    osb = stp.tile([128, 1024], FP16, tag="osbf", bufs=3,
                                   name=f"osbf{b}_{ob}")
                    if ob % 3 == 0:
                        nc.vector.tensor_copy(osb[:], w[:])
                    else:
                        nc.scalar.copy(osb[:], w[:])
                    nc.sync.dma_start(out_d[trows, :], osb[:])

                prev = None          # head whose PV bursts run during cur QK
                phase3_todo = []
                a2_of = {}           # b -> list of A2 tiles per query block
                for b in range(B):
                    for h in range(HPC):
                        hr = slice(h * 64, (h + 1) * 64)
                        PT = bigp.tile([128, NTB, T], FP16, tag="PT",
                                       name=f"PT{b}{h}")
                        if h == 0:
                            a2_of[b] = [
                                a2p.tile([128, 128], FP16, tag="A2",
                                         bufs=2 * NTB, name=f"A2_{b}_{tb}")
                                for tb in range(NTB)
                            ]
                        A2s = a2_of[b]
                        last = (b == B - 1 and h == HPC - 1)
                        cur = {"b": b, "h": h, "A2": A2s}
                        if last:
                            phase3_todo.extend((b, ob) for ob in range(NTB))
                        for tb in range(NTB):
                            done_this_tb = []
                            if b == 0 and h == 0 and tb == 1:
                                emit_phase1(2)
                            if b == 0 and h == 0 and tb == 3:
                                emit_phase1(3)
                            if prev is not None and tb in (2, 6, 10, 14):
                                emit_pv_burst(prev, tb // 4)
                            if last and tb in (5, 9, 13):
                                cur["PT"] = PT
                                emit_pv_burst(cur, (tb - 5) // 4)
                            tcols = slice(b * T + tb * 128,
                                          b * T + (tb + 1) * 128)
                            Sa = spool.tile([128, 1024], F32, tag="S",
                                            name=f"Sa{b}{h}{tb}")
                            Sb = spool.tile([128, 1024], F32, tag="S",
                                            name=f"Sb{b}{h}{tb}")
                            for j in range(4):
                                S = (Sa if j < 2 else Sb)
                                scols = slice(b * T + j * 512,
                                              b * T + (j + 1) * 512)
                                nc.tensor.matmul(
                                    S[:, (j % 2) * 512:(j % 2 + 1) * 512],
                                    QT[hr, tcols], KT[hr, scols], start=True,
                                    stop=True)
                            # row max: negated reduce per half on DVE (a DVE
                            # op may read only ONE PSUM operand), min-combine
                            # on Pool (SBUF-only engine)
                            mneg = stp.tile([128, 2], FP16, tag="mneg", bufs=4,
                                            name=f"mneg{b}{h}{tb}")
                            nc.vector.reduce_max(mneg[:, 0:1], Sa[:], axis=AX,
                                                 negate=True)
                            nc.vector.reduce_max(mneg[:, 1:2], Sb[:], axis=AX,
                                                 negate=True)
                            negb = stp.tile([128, 1], FP16, tag="negb", bufs=4,
                                            name=f"negb{b}{h}{tb}")
                            nc.vector.tensor_reduce(negb[:], mneg[:], AX,
                                                    mybir.AluOpType.min)
                            # exp (unnormalized); halves so the transpose of
                            # half a overlaps the exp of half b
                            Pa = ptp.tile([128, 1024], FP16, tag="Pt",
                                          name=f"Pa{b}{h}{tb}")
                            Pb = ptp.tile([128, 1024], FP16, tag="Pt",
                                          name=f"Pb{b}{h}{tb}")
                            nc.scalar.activation(Pa[:], Sa[:], EXP,
                                                 bias=negb[:], scale=1.0)
                            nc.sync.dma_start_transpose(
                                PT[:, 0:8, tb * 128:(tb + 1) * 128], Pa[:])
                            nc.scalar.activation(Pb[:], Sb[:], EXP,
                                                 bias=negb[:], scale=1.0)
                            nc.sync.dma_start_transpose(
                                PT[:, 8:16, tb * 128:(tb + 1) * 128], Pb[:])
                            if last:
                                # own-batch blocks: only those whose PV
                                # burst has already been emitted
                                while (tb >= 7 and phase3_todo
                                       and phase3_todo[0][1]
                                       < 4 * ((tb - 5) // 4 + 1)
                                       and len(done_this_tb) < 2):
                                    done_this_tb.append(1)
                                    emit_phase3_block_spool(*phase3_todo.pop(0))
                            elif (tb >= 3 or h == 1) and phase3_todo:
                                emit_phase3_block_spool(*phase3_todo.pop(0))
                        if prev is not None and prev["h"] == HPC - 1 and not last:
                            phase3_todo.extend(
                                (prev["b"], ob) for ob in range(NTB))
                        prev = {"b": b, "h": h, "PT": PT, "A2": A2s}

                # flush: last PV burst, then the remaining phase-3 blocks
                emit_pv_burst(prev, 3)
                for blk in phase3_todo:
                    emit_phase3_block_spool(*blk)
                phase3_todo = []

    nc.compile()
    return nc


def make_in_maps(x, Wq, Wk, Wv, Wo):
    """Build the 8 per-core input maps from the full inputs."""
    x = np.asarray(x, np.float32)
    Wq = np.asarray(Wq, np.float32)
    Wk = np.asarray(Wk, np.float32)
    Wv = np.asarray(Wv, np.float32)
    Wo = np.asarray(Wo, np.float32)

    xt = np.ascontiguousarray(x.reshape(BT, E).T).astype(NF16)   # [E, BT]
    in_maps = []
    for c in range(NCORES):
        hsl = slice(c * HPC, (c + 1) * HPC)

        def _pmaj(w):  # [E, HI] -> [128, EC, HI] (partition-major)
            return np.ascontiguousarray(
                w.reshape(EC, 128, HI).transpose(1, 0, 2)).astype(NF16)

        m = {
            "xt": xt,
            "wq": _pmaj(np.concatenate(list(Wq[hsl]), axis=1)),
            "wk": _pmaj(np.concatenate(list(Wk[hsl]), axis=1)),
            "wv": _pmaj(np.concatenate(list(Wv[hsl]), axis=1)),
            "wo_t": np.ascontiguousarray(
                Wo[:, c * HI:(c + 1) * HI].T).astype(NF16),
        }
        in_maps.append(m)
    return in_maps


_CACHED = {}


def _get_program() -> bass.Bass:
    if "p" not in _CACHED:
        _CACHED["p"] = build_program()
    return _CACHED["p"]


def kernel(**inputs) -> np.ndarray:
    from concourse.bass_utils import run_bass_kernel_spmd

    nc = _get_program()
    in_maps = make_in_maps(inputs["x"], inputs["Wq"], inputs["Wk"],
                           inputs["Wv"], inputs["Wo"])
    res = run_bass_kernel_spmd(nc, in_maps, core_ids=list(range(NCORES)))
    out = np.zeros((BT, E), np.float32)
    for c in range(NCORES):
        out += np.asarray(res.results[c]["out"], np.float32)
    return out.reshape(B, T, E)


# revision 51
# speedup vs baseline: 1.7367x; 1.0560x over previous
"""Tensor-parallel multi-head attention for Trainium2 (8 NeuronCores).

Problem: B=2, T=2048, E=1024, H=16 heads of dim 64.
  q/k/v = einsum('hei,bte->hbti'); s = q@k^T/sqrt(T); p = softmax(s)
  att = p@v; out = concat_heads(att) @ Wo^T

Sharding: tensor-parallel over heads - 2 heads per core. Each core computes
its heads' attention plus its slice of the output projection (Wo sharded
along its input axis); partial outputs are summed across cores.

v2 layout (vs the v1 8x[128,512] ring design):
  - ONE unified PSUM ring: 4 slots x [128,1024] (2 banks each) shared by
    the S tiles, the PV burst accumulators, and whole phase-3 blocks, so
    consecutive query blocks pipeline without a dedicated-bank squeeze.
  - Per query block: 4 QK matmuls into two S halves; row max = two
    negated DVE reduces (a DVE op may read only ONE PSUM operand) + a
    tiny min-combine; exp = 2x[128,1024] ACT instructions, each followed
    by its own [128,1024] DMA transpose into PT (s-major).
  - PV is reoriented: out[t, i] accumulates over the 16 s-chunks with
    lhsT = PT tiles (stationary) and rhs = V [s,65] (64 data cols + a
    ones column -> den lands per-partition-t at col 64 of the same PSUM
    region; 4 query blocks' accumulators pack into one bank, riding a
    single PSUM zero-region group: only the first matmul starts it).
  - Normalization on the evac: ACT Copy with per-partition scale=1/den
    into A2 [t, (h i)] tiles; one [128,128] DMA transpose per query
    block assembles OT [hi, t] for the out projection.
  - Phase 3 = whole [128,1024] blocks in one ring slot (2 matmuls,
    1 evac, 1 out DMA); the last head's PV bursts run inside its own
    softmax loop so the tail flush is just one burst + 4 blocks.
  - All DMAs issue from the SP HWDGE queue. Pool/GPSIMD is avoided:
    it cannot touch PSUM, and strided SBUF copies/memsets on it
    miscompute on real HW (sim-only success).

Phase 1 is projection-major (each projection's 16 matmuls + evac hold a
ring slot ~4us, letting early attention blocks interleave); batch 1's
projections are emitted inside (b=0,h=0)'s first query blocks.

Engine budget per core-iteration (cost model): DVE 184 (row maxes +
QT/KT evacs), DMA 175 (64 P transposes = 115), ACT 173 (128 exps + A2
evacs), PE 140 (QKV 41 + QK 55 + PV 28 + out-proj 14). Cost-model span
282us vs 351us for the v1 baseline.
"""

import sys

sys.path.insert(0, "/opt/trn_rl_repo")

import numpy as np

import concourse.bass as bass
import concourse.mybir as mybir
import concourse.tile as tile
from concourse import bacc

NF16 = np.float16

B, T, E = 2, 2048, 1024
H, I = 16, 64
NCORES = 8
HPC = H // NCORES            # heads per core = 2
BT = B * T                   # 4096
HI = HPC * I                 # 128 = per-core slice of the h*i axis
EC = E // 128                # 8 e-chunks
NTB = T // 128               # 16 query blocks per batch
SCALE = 1.0 / float(np.sqrt(np.float32(T)))

F32 = mybir.dt.float32
FP16 = mybir.dt.float16
AX = mybir.AxisListType.X
MUL = mybir.AluOpType.mult
ADD = mybir.AluOpType.add
MAX = mybir.AluOpType.max
EXP = mybir.ActivationFunctionType.Exp

ABLATE_DEFAULT = ""


def build_program(repeat: int = 1, ablate: str | None = None) -> bass.Bass:
    if ablate is None:
        ablate = ABLATE_DEFAULT
    nc = bacc.Bacc("TRN2", target_bir_lowering=False, debug=False,
                   num_devices=NCORES)

    xt_d = nc.dram_tensor("xt", [E, BT], FP16, kind="ExternalInput")
    wq_d = nc.dram_tensor("wq", [128, EC, HI], FP16, kind="ExternalInput")
    wk_d = nc.dram_tensor("wk", [128, EC, HI], FP16, kind="ExternalInput")
    wv_d = nc.dram_tensor("wv", [128, EC, HI], FP16, kind="ExternalInput")
    wo_d = nc.dram_tensor("wo_t", [HI, E], FP16, kind="ExternalInput")
    out_d = nc.dram_tensor("out", [BT, E], FP16, kind="ExternalOutput")

    with tile.TileContext(nc) as tc:
        with (
            tc.tile_pool(name="sp", bufs=4, space="PSUM") as spool,
            tc.tile_pool(name="wp", bufs=1) as wp,
            tc.tile_pool(name="xp", bufs=4) as xp,
            tc.tile_pool(name="pk", bufs=1) as pk,
            tc.tile_pool(name="big", bufs=2) as bigp,
            tc.tile_pool(name="ptp", bufs=4) as ptp,
            tc.tile_pool(name="a2p", bufs=2) as a2p,
            tc.tile_pool(name="stp", bufs=2) as stp,
        ):
            wq = wp.tile([128, EC, HI], FP16, tag="wq")
            wk = wp.tile([128, EC, HI], FP16, tag="wk")
            wv = wp.tile([128, EC, HI], FP16, tag="wv")
            wo = wp.tile([128, E], FP16, tag="wo")
            nc.sync.dma_start(wq[:], wq_d[:])
            nc.sync.dma_start(wk[:], wk_d[:])
            nc.sync.dma_start(wv[:], wv_d[:])
            nc.sync.dma_start(wo[:], wo_d[:])

            for _rep in range(repeat):
                QT = pk.tile([128, BT], FP16, tag="QT")
                KT = pk.tile([128, BT], FP16, tag="KT")
                # V: per s-chunk, 64 data cols + ones col per head (col 64 /
                # col 129): the PV matmul streams V as rhs and the ones col
                # accumulates den = sum_s P[t,s] into the same PSUM tile.
                V = pk.tile([128, BT // 128, 130], FP16, tag="V")
                OT = pk.tile([128, BT], FP16, tag="OT")
                nc.vector.memset(V[:, :, 64:65], 1.0)
                nc.vector.memset(V[:, :, 129:130], 1.0)

                # ---------- Phase 1: QKV projections (1024-token pairs) -----
                x_held = {}

                def load_x(bp):
                    cols = slice(bp * 1024, (bp + 1) * 1024)
                    xcs = []
                    for g in range(4):
                        xg = xp.tile([128, 2, 1024], FP16, tag="x",
                                     name=f"x{bp}_{g}")
                        nc.sync.dma_start(
                            xg[:], xt_d[g * 256:(g + 1) * 256, cols].rearrange(
                                "(o p) t -> p o t", p=128))
                        xcs.append(xg)
                    x_held[bp] = xcs
                    return xcs

                def emit_proj(bp, which):
                    # projection-major: each projection's 16 matmuls finish
                    # and evac before the next starts, so its ring slot is
                    # held ~4us instead of the whole block-pair's ~13us
                    xcs = x_held.get(bp) or load_x(bp)
                    hcols = slice(bp * 1024, (bp + 1) * 1024)
                    for w_, nm in which:
                        ps = spool.tile([128, 1024], F32, tag="S",
                                        name=f"p{nm}{bp}")
                        for ec in range(EC):
                            xc = xcs[ec // 2][:, ec % 2, :]
                            st = (ec == 0)
                            sp = (ec == EC - 1)
                            for half in range(2):
                                nc.tensor.matmul(
                                    ps[:, half * 512:(half + 1) * 512],
                                    w_[:, ec, :],
                                    xc[:, half * 512:(half + 1) * 512],
                                    start=st, stop=sp)
                        if w_ is wq:
                            nc.vector.tensor_scalar_mul(QT[:, hcols], ps[:],
                                                        SCALE)
                        elif w_ is wk:
                            nc.vector.tensor_copy(KT[:, hcols], ps[:])
                        else:
                            vsb = ptp.tile([128, 1024], FP16, tag="Pt",
                                           name=f"vsb{bp}")
                            nc.scalar.copy(vsb[:], ps[:])
                            stag = ptp.tile([128, 8, 128], FP16, tag="Pt",
                                            name=f"stag{bp}")
                            nc.sync.dma_start_transpose(stag[:], vsb[:])
                            nc.vector.tensor_copy(
                                V[:, bp * 8:(bp + 1) * 8, 0:64],
                                stag[:, :, 0:64])
                            nc.vector.tensor_copy(
                                V[:, bp * 8:(bp + 1) * 8, 65:129],
                                stag[:, :, 64:128])

                def emit_phase1(bp):
                    emit_proj(bp, ((wq, "q"), (wk, "k"), (wv, "v")))
                    x_held.pop(bp, None)

                # batch 0's tokens (bp 0,1) upfront; bp 2,3 interleave into
                # the early query blocks of (b=0, h=0)
                emit_phase1(0)
                emit_phase1(1)

                # ---------- Phase 2 + PV bursts / phase 3 -------------------
                def emit_pv_burst(prev, g, width=4):
                    # PV of prev head for query blocks width*g.., reoriented:
                    # out[t,i] accumulates over the 16 s-chunks; lhsT = PT
                    # tile [s,t-block], rhs = V [s, 65] (ones col -> den at
                    # col 64). One PSUM bank holds the blocks' accumulators.
                    pb, ph = prev["b"], prev["h"]
                    vw = 65
                    vcols = slice(ph * vw, (ph + 1) * vw)
                    ob = spool.tile([128, width, vw], F32, tag="S",
                                    name=f"ob{pb}{ph}{g}w{width}")
                    # start=True zeroes the whole 2KB PSUM zero region (the
                    # bank), so only the first matmul starts the group and
                    # only the last one stops it; the packed accumulator
                    # regions ride the same group.
                    for sc in range(NTB):
                        rhs = V[:, pb * NTB + sc, vcols]
                        for j in range(width):
                            tb = width * g + j
                            nc.tensor.matmul(
                                ob[:, j, :],
                                prev["PT"][:, sc, tb * 128:(tb + 1) * 128],
                                rhs, start=(sc == 0 and j == 0),
                                stop=(sc == NTB - 1 and j == width - 1))
                    # reciprocal of the dens (strided col 64), then evac
                    # with per-partition normalize into A2[t, ph*64+i]
                    rcp = stp.tile([128, width], F32, tag="rcp", bufs=3,
                                   name=f"rcp{pb}{ph}{g}w{width}")
                    nc.vector.reciprocal(rcp[:], ob[:, :, 64])
                    for j in range(width):
                        tb = width * g + j
                        A2 = prev["A2"][tb]
                        # normalize on the evac: ACT Copy with per-partition
                        # scale = 1/den
                        nc.scalar.activation(
                            A2[:, ph * 64:(ph + 1) * 64], ob[:, j, 0:64],
                            mybir.ActivationFunctionType.Copy,
                            scale=rcp[:, j:j + 1])
                        if ph == HPC - 1:
                            # both heads done for this query block: OT cols
                            nc.sync.dma_start_transpose(
                                OT[:, pb * T + tb * 128:pb * T + (tb + 1) * 128],
                                A2[:])

                def emit_phase3_block_spool(b, ob):
                    # whole [128,1024] block in one 2-bank ring slot
                    # (2 matmuls, 1 evac, 1 out DMA)
                    trows = slice(b * T + ob * 128, b * T + (ob + 1) * 128)
                    w = spool.tile([128, 1024], F32, tag="S",
                                   name=f"wf{b}_{ob}")
                    for half in range(2):
                        nc.tensor.matmul(w[:, half * 512:(half + 1) * 512],
                                         OT[:, trows],
                                         wo[:, half * 512:(half + 1) * 512],
                                         start=True, stop=True)
                    osb = stp.tile([128, 1024], FP16, tag="osbf", bufs=3,
                                   name=f"osbf{b}_{ob}")
                    if ob % 3 == 0:
                        nc.vector.tensor_copy(osb[:], w[:])
                    else:
                        nc.scalar.copy(osb[:], w[:])
                    nc.sync.dma_start(out_d[trows, :], osb[:])

                prev = None          # head whose PV bursts run during cur QK
                phase3_todo = []
                a2_of = {}           # b -> list of A2 tiles per query block
                for b in range(B):
                    for h in range(HPC):
                        hr = slice(h * 64, (h + 1) * 64)
                        PT = bigp.tile([128, NTB, T], FP16, tag="PT",
                                       name=f"PT{b}{h}")
                        if h == 0:
                            a2_of[b] = [
                                a2p.tile([128, 128], FP16, tag="A2",
                                         bufs=2 * NTB, name=f"A2_{b}_{tb}")
                                for tb in range(NTB)
                            ]
                        A2s = a2_of[b]
                        last = (b == B - 1 and h == HPC - 1)
                        cur = {"b": b, "h": h, "A2": A2s}
                        if last:
                            phase3_todo.extend((b, ob) for ob in range(NTB))
                        for tb in range(NTB):
                            done_this_tb = []
                            if b == 0 and h == 0 and tb == 1:
                                emit_phase1(2)
                            if b == 0 and h == 0 and tb == 3:
                                emit_phase1(3)
                            if prev is not None and tb in (2, 6, 10, 14):
                                emit_pv_burst(prev, tb // 4)
                            if last and tb in (5, 9, 13):
                                cur["PT"] = PT
                                emit_pv_burst(cur, (tb - 5) // 4)
                            tcols = slice(b * T + tb * 128,
                                          b * T + (tb + 1) * 128)
                            Sa = spool.tile([128, 1024], F32, tag="S",
                                            name=f"Sa{b}{h}{tb}")
                            Sb = spool.tile([128, 1024], F32, tag="S",
                                            name=f"Sb{b}{h}{tb}")
                            for j in range(4):
                                S = (Sa if j < 2 else Sb)
                                scols = slice(b * T + j * 512,
                                              b * T + (j + 1) * 512)
                                nc.tensor.matmul(
                                    S[:, (j % 2) * 512:(j % 2 + 1) * 512],
                                    QT[hr, tcols], KT[hr, scols], start=True,
                                    stop=True)
                            # row max: negated reduce per half on DVE (a DVE
                            # op may read only ONE PSUM operand), min-combine
                            # on Pool (SBUF-only engine)
                            mneg = stp.tile([128, 2], FP16, tag="mneg", bufs=6,
                                            name=f"mneg{b}{h}{tb}")
                            nc.vector.reduce_max(mneg[:, 0:1], Sa[:], axis=AX,
                                                 negate=True)
                            nc.vector.reduce_max(mneg[:, 1:2], Sb[:], axis=AX,
                                                 negate=True)
                            negb = stp.tile([128, 1], FP16, tag="negb", bufs=6,
                                            name=f"negb{b}{h}{tb}")
                            nc.vector.tensor_reduce(negb[:], mneg[:], AX,
                                                    mybir.AluOpType.min)
                            # exp (unnormalized); halves so the transpose of
                            # half a overlaps the exp of half b
                            Pa = ptp.tile([128, 1024], FP16, tag="Pt",
                                          name=f"Pa{b}{h}{tb}")
                            Pb = ptp.tile([128, 1024], FP16, tag="Pt",
                                          name=f"Pb{b}{h}{tb}")
                            nc.scalar.activation(Pa[:], Sa[:], EXP,
                                                 bias=negb[:], scale=1.0)
                            nc.sync.dma_start_transpose(
                                PT[:, 0:8, tb * 128:(tb + 1) * 128], Pa[:])
                            nc.scalar.activation(Pb[:], Sb[:], EXP,
                                                 bias=negb[:], scale=1.0)
                            nc.sync.dma_start_transpose(
                                PT[:, 8:16, tb * 128:(tb + 1) * 128], Pb[:])
                            if last:
                                # own-batch blocks: only those whose PV
                                # burst has already been emitted
                                while (tb >= 7 and phase3_todo
                                       and phase3_todo[0][1]
                                       < 4 * ((tb - 5) // 4 + 1)
                                       and len(done_this_tb) < 2):
                                    done_this_tb.append(1)
                                    emit_phase3_block_spool(*phase3_todo.pop(0))
                            elif (tb >= 3 or h == 1) and phase3_todo:
                                emit_phase3_block_spool(*phase3_todo.pop(0))
                        if prev is not None and prev["h"] == HPC - 1 and not last:
                            phase3_todo.extend(
                                (prev["b"], ob) for ob in range(NTB))
                        prev = {"b": b, "h": h, "PT": PT, "A2": A2s}

                # flush: last PV burst, then the remaining phase-3
                emit_pv_burst(prev, 3)
                for blk in phase3_todo:
                    emit_phase3_block_spool(*blk)
                phase3_todo = []

    nc.compile()
    return nc


def make_in_maps(x, Wq, Wk, Wv, Wo):
    """Build the 8 per-core input maps from the full inputs."""
    x = np.asarray(x, np.float32)
    Wq = np.asarray(Wq, np.float32)
    Wk = np.asarray(Wk, np.float32)
    Wv = np.asarray(Wv, np.float32)
    Wo = np.asarray(Wo, np.float32)

    xt = np.ascontiguousarray(x.reshape(BT, E).T).astype(NF16)   # [E, BT]
    in_maps = []
    for c in range(NCORES):
        hsl = slice(c * HPC, (c + 1) * HPC)

        def _pmaj(w):  # [E, HI] -> [128, EC, HI] (partition-major)
            return np.ascontiguousarray(
                w.reshape(EC, 128, HI).transpose(1, 0, 2)).astype(NF16)

        m = {
            "xt": xt,
            "wq": _pmaj(np.concatenate(list(Wq[hsl]), axis=1)),
            "wk": _pmaj(np.concatenate(list(Wk[hsl]), axis=1)),
            "wv": _pmaj(np.concatenate(list(Wv[hsl]), axis=1)),
            "wo_t": np.ascontiguousarray(
                Wo[:, c * HI:(c + 1) * HI].T).astype(NF16),
        }
        in_maps.append(m)
    return in_maps


_CACHED = {}


def _get_program() -> bass.Bass:
    if "p" not in _CACHED:
        _CACHED["p"] = build_program()
    return _CACHED["p"]


def kernel(**inputs) -> np.ndarray:
    from concourse.bass_utils import run_bass_kernel_spmd

    nc = _get_program()
    in_maps = make_in_maps(inputs["x"], inputs["Wq"], inputs["Wk"],
                           inputs["Wv"], inputs["Wo"])
    res = run_bass_kernel_spmd(nc, in_maps, core_ids=list(range(NCORES)))
    out = np.zeros((BT, E), np.float32)
    for c in range(NCORES):
        out += np.asarray(res.results[c]["out"], np.float32)
    return out.reshape(B, T, E)
